# revision 7
# baseline (speedup 1.0000x reference)
"""LIF neuron scan kernel for Trainium2 (Bass/Tile), SPMD over 8 NeuronCores.

Reference computation (T=32, B=16, N=65536, f32):
    m = 0
    for t in range(T):
        m = 0.25 * m + x[t]          # membrane update (beta = 0.25)
        spike[t] = (m >= 1.0)        # heaviside
        membrane[t] = m              # recorded pre-reset
        m = m - spike[t]             # soft reset (threshold = 1.0)
    return spikes, membranes

Sharding: split N across the 8 cores (N/8 = 8192 per core). The scan
recurrence is over T only, so each core runs an independent sequential
scan over its (T, B, 8192) slice with zero communication.

Per-core layout: the (B=16, 8192) plane per timestep flattens to
(128, 1024) — partition dim 128, 1024 contiguous f32 per partition
(4 KiB DMA lines). Timesteps are processed in blocks of TB=4; block
transfers are split so each DMA is 0.5-1 MiB (past the DMA setup knee
while keeping the pipeline fine-grained).

Spikes travel as uint8 on the wire (0/1 exact) and are converted back
to f32 on the host, cutting output bytes by 3/8. Membranes stay f32 —
the scan is bit-exact vs the f32 reference.

All DMA is HWDGE: input loads on the SP ring, output stores on the ACT
ring, so loads are never queued behind stores.
"""

import os

import numpy as np

import concourse.bacc as bacc
import concourse.mybir as mybir
import concourse.tile as tile
from concourse.bass_utils import run_bass_kernel_spmd
from concourse.mybir import AluOpType

BETA = 0.25
THRESHOLD = 1.0

T, B, N = 32, 16, 65536
NCORES = 8
NS = N // NCORES          # 8192 columns per core
P = 128                   # SBUF partitions
F = (B * NS) // P         # 1024 free-dim elements per partition
TB = 4                    # timesteps per SBUF block
SPLIT_X = 2               # x-load pieces per block   (2 -> 1 MiB per DMA)
SPLIT_MEM = 2             # membrane-store pieces     (2 -> 1 MiB per DMA)

_cache = {}


def _build_nc():
    nc = bacc.Bacc("TRN2", target_bir_lowering=False, debug=False)
    f32 = mybir.dt.float32
    u8 = mybir.dt.uint8
    x_d = nc.dram_tensor("x", [T, P, F], f32, kind="ExternalInput").ap()
    spk_d = nc.dram_tensor("spikes", [T, P, F], u8, kind="ExternalOutput").ap()
    mem_d = nc.dram_tensor("membranes", [T, P, F], f32, kind="ExternalOutput").ap()

    with tile.TileContext(nc) as tc:
        with (
            tc.tile_pool(name="xin", bufs=4) as xp,
            tc.tile_pool(name="mstate", bufs=1) as mp,
            tc.tile_pool(name="mem", bufs=3) as memp,
            tc.tile_pool(name="spk", bufs=3) as spkp,
        ):
            m = mp.tile([P, F], f32)
            for blk in range(T // TB):
                t0 = blk * TB
                xt = xp.tile([P, TB * F], f32)
                xstep = TB // SPLIT_X
                for s in range(0, TB, xstep):
                    nc.sync.dma_start(
                        xt[:, s * F : (s + xstep) * F].rearrange(
                            "p (t f) -> p t f", t=xstep
                        ),
                        x_d[t0 + s : t0 + s + xstep].rearrange("t p f -> p t f"),
                    )
                mem = memp.tile([P, TB * F], f32)
                spk = spkp.tile([P, TB * F], u8)
                for i in range(TB):
                    t = t0 + i
                    sl = slice(i * F, (i + 1) * F)
                    if t == 0:
                        # m starts at 0: first pre-reset membrane is x[0].
                        mpre = xt[:, sl]
                        nc.scalar.copy(mem[:, sl], xt[:, sl])
                    else:
                        mpre = mem[:, sl]
                        nc.vector.scalar_tensor_tensor(
                            mpre, m[:], BETA, xt[:, sl],
                            AluOpType.mult, AluOpType.add,
                        )
                    nc.vector.tensor_scalar(
                        spk[:, sl], mpre, THRESHOLD, None, AluOpType.is_ge
                    )
                    nc.vector.tensor_tensor(
                        m[:], mpre, spk[:, sl], AluOpType.subtract
                    )
                mstep = TB // SPLIT_MEM
                for s in range(0, TB, mstep):
                    nc.scalar.dma_start(
                        mem_d[t0 + s : t0 + s + mstep].rearrange("t p f -> p t f"),
                        mem[:, s * F : (s + mstep) * F].rearrange(
                            "p (t f) -> p t f", t=mstep
                        ),
                    )
                nc.scalar.dma_start(
                    spk_d[t0 : t0 + TB].rearrange("t p f -> p t f"),
                    spk[:].rearrange("p (t f) -> p t f", t=TB),
                )
    nc.finalize()  # run Bacc passes (reg alloc, sync-wait splitting)
    return nc


last_results = None  # BassKernelResults of the most recent run (for profiling)


def kernel(x: np.ndarray):
    global last_results
    assert x.shape == (T, B, N) and x.dtype == np.float32

    if "nc" not in _cache:
        _cache["nc"] = _build_nc()
    nc = _cache["nc"]

    in_maps = [
        {"x": np.ascontiguousarray(x[:, :, c * NS : (c + 1) * NS]).reshape(T, P, F)}
        for c in range(NCORES)
    ]
    res = run_bass_kernel_spmd(
        nc,
        in_maps,
        core_ids=list(range(NCORES)),
        trace=bool(int(os.environ.get("LIF_TRACE", "0"))),
    )
    last_results = res

    spikes = np.empty((T, B, N), dtype=np.float32)
    membranes = np.empty((T, B, N), dtype=np.float32)
    for c in range(NCORES):
        spikes[:, :, c * NS : (c + 1) * NS] = (
            res.results[c]["spikes"].astype(np.float32).reshape(T, B, NS)
        )
        membranes[:, :, c * NS : (c + 1) * NS] = res.results[c]["membranes"].reshape(
            T, B, NS
        )
    return spikes, membranes


# revision 9
# speedup vs baseline: 1.0184x; 1.0184x over previous
"""LIF neuron scan kernel for Trainium2 (Bass/Tile), SPMD over 8 NeuronCores.

Reference computation (T=32, B=16, N=65536, f32):
    m = 0
    for t in range(T):
        m = 0.25 * m + x[t]          # membrane update (beta = 0.25)
        spike[t] = (m >= 1.0)        # heaviside
        membrane[t] = m              # recorded pre-reset
        m = m - spike[t]             # soft reset (threshold = 1.0)
    return spikes, membranes

Sharding: split N across the 8 cores (N/8 = 8192 per core). The scan
recurrence is over T only, so each core runs an independent sequential
scan over its (T, B, 8192) slice with zero communication.

Per-core layout: the (B=16, 8192) plane per timestep flattens to
(128, 1024) — partition dim 128, 1024 contiguous f32 per partition
(4 KiB DMA lines). Timesteps are processed in blocks of TB=4; block
transfers are split so each DMA is 0.5-1 MiB (past the DMA setup knee
while keeping the pipeline fine-grained).

Spikes travel as uint8 on the wire (0/1 exact) and are converted back
to f32 on the host, cutting output bytes by 3/8. Membranes stay f32 —
the scan is bit-exact vs the f32 reference.

All DMA is HWDGE: input loads on the SP ring, output stores on the ACT
ring, so loads are never queued behind stores.
"""

import os

import numpy as np

import concourse.bacc as bacc
import concourse.mybir as mybir
import concourse.tile as tile
from concourse.bass_utils import run_bass_kernel_spmd
from concourse.mybir import AluOpType

BETA = 0.25
THRESHOLD = 1.0

T, B, N = 32, 16, 65536
NCORES = 8
NS = N // NCORES          # 8192 columns per core
P = 128                   # SBUF partitions
F = (B * NS) // P         # 1024 free-dim elements per partition
TB = 4                    # timesteps per SBUF block
SPLIT_X = 4               # x-load pieces per block   (4 -> 512 KiB per DMA)
SPLIT_MEM = 4             # membrane-store pieces     (4 -> 512 KiB per DMA)

_cache = {}


def _build_nc():
    nc = bacc.Bacc("TRN2", target_bir_lowering=False, debug=False)
    f32 = mybir.dt.float32
    u8 = mybir.dt.uint8
    x_d = nc.dram_tensor("x", [T, P, F], f32, kind="ExternalInput").ap()
    spk_d = nc.dram_tensor("spikes", [T, P, F], u8, kind="ExternalOutput").ap()
    mem_d = nc.dram_tensor("membranes", [T, P, F], f32, kind="ExternalOutput").ap()

    with tile.TileContext(nc) as tc:
        with (
            tc.tile_pool(name="xin", bufs=4) as xp,
            tc.tile_pool(name="mstate", bufs=1) as mp,
            tc.tile_pool(name="mem", bufs=3) as memp,
            tc.tile_pool(name="spk", bufs=3) as spkp,
        ):
            m = mp.tile([P, F], f32)
            for blk in range(T // TB):
                t0 = blk * TB
                xt = xp.tile([P, TB * F], f32)
                xstep = TB // SPLIT_X
                for s in range(0, TB, xstep):
                    nc.sync.dma_start(
                        xt[:, s * F : (s + xstep) * F].rearrange(
                            "p (t f) -> p t f", t=xstep
                        ),
                        x_d[t0 + s : t0 + s + xstep].rearrange("t p f -> p t f"),
                    )
                mem = memp.tile([P, TB * F], f32)
                spk = spkp.tile([P, TB * F], u8)
                for i in range(TB):
                    t = t0 + i
                    sl = slice(i * F, (i + 1) * F)
                    if t == 0:
                        # m starts at 0: first pre-reset membrane is x[0].
                        mpre = xt[:, sl]
                        nc.scalar.copy(mem[:, sl], xt[:, sl])
                    else:
                        mpre = mem[:, sl]
                        nc.vector.scalar_tensor_tensor(
                            mpre, m[:], BETA, xt[:, sl],
                            AluOpType.mult, AluOpType.add,
                        )
                    nc.vector.tensor_scalar(
                        spk[:, sl], mpre, THRESHOLD, None, AluOpType.is_ge
                    )
                    nc.vector.tensor_tensor(
                        m[:], mpre, spk[:, sl], AluOpType.subtract
                    )
                mstep = TB // SPLIT_MEM
                for s in range(0, TB, mstep):
                    nc.scalar.dma_start(
                        mem_d[t0 + s : t0 + s + mstep].rearrange("t p f -> p t f"),
                        mem[:, s * F : (s + mstep) * F].rearrange(
                            "p (t f) -> p t f", t=mstep
                        ),
                    )
                nc.scalar.dma_start(
                    spk_d[t0 : t0 + TB].rearrange("t p f -> p t f"),
                    spk[:].rearrange("p (t f) -> p t f", t=TB),
                )
    nc.finalize()  # run Bacc passes (reg alloc, sync-wait splitting)
    return nc


last_results = None  # BassKernelResults of the most recent run (for profiling)


def kernel(x: np.ndarray):
    global last_results
    assert x.shape == (T, B, N) and x.dtype == np.float32

    if "nc" not in _cache:
        _cache["nc"] = _build_nc()
    nc = _cache["nc"]

    in_maps = [
        {"x": np.ascontiguousarray(x[:, :, c * NS : (c + 1) * NS]).reshape(T, P, F)}
        for c in range(NCORES)
    ]
    trace = bool(int(os.environ.get("LIF_TRACE", "0")))
    if not trace:
        # NTFF tracing needs antenv.axon_hooks, which this container does
        # not ship — make sure a stray BASS_TRACE=1 can't crash the run.
        os.environ["BASS_NEVER_TRACE"] = "1"
    res = run_bass_kernel_spmd(
        nc,
        in_maps,
        core_ids=list(range(NCORES)),
        trace=trace,
    )
    last_results = res

    spikes = np.empty((T, B, N), dtype=np.float32)
    membranes = np.empty((T, B, N), dtype=np.float32)
    for c in range(NCORES):
        spikes[:, :, c * NS : (c + 1) * NS] = (
            res.results[c]["spikes"].astype(np.float32).reshape(T, B, NS)
        )
        membranes[:, :, c * NS : (c + 1) * NS] = res.results[c]["membranes"].reshape(
            T, B, NS
        )
    return spikes, membranes
